# revision 5
# baseline (speedup 1.0000x reference)
"""Trainium2 Bass kernel for nn_AE_RNN (2-layer GRU AE, scan over T, scalar loss).

Strategy: data-parallel over batch across 8 NeuronCores (128 rows/core), no
collectives (host sums the 8 partial losses).  On each core, feature-major
layout [feature, t*128 + b]:
  - bulk phi_u MLP precompute per time-chunk,
  - serial GRU scan (gates in PSUM, sigmoid/tanh on ScalarE, fused DVE ops),
  - measurement branch (dynn -> x_mean/logvar -> phi_x -> menn + C) pipelined
    4 timesteps at a time behind the scan,
  - loss as sum(yhat^2) - 2*sum(yhat*y) on-chip + sum(y^2) on host.
"""

import numpy as np
import ml_dtypes

import concourse.bass as bass
import concourse.tile as tile
from concourse import bacc, mybir
from concourse.bass_utils import run_bass_kernel_spmd

B, T, UD, YD, ZD, HD, L = 1024, 1024, 16, 16, 16, 128, 2
NCORES = 8
BL = B // NCORES  # 128

BF = mybir.dt.bfloat16
F32 = mybir.dt.float32
AF = mybir.ActivationFunctionType
OP = mybir.AluOpType
BF_NP = ml_dtypes.bfloat16


def build(T_total=T, Tc=64):
    """Build the per-core Bass graph.  Returns (nc, meta)."""
    assert T_total % Tc == 0 and Tc % 4 == 0
    NCH = T_total // Tc          # number of time chunks
    NG = Tc // 4                 # measurement / bulk groups per chunk
    NGT = T_total // 4           # total groups
    GW = 4 * BL                  # group width in columns (512)

    nc = bacc.Bacc("TRN2", target_bir_lowering=False, debug=False)

    def param(name, shape, dt=BF):
        return nc.declare_dram_parameter(name, list(shape), dt, isOutput=False)

    u_p = param("u", [UD, T_total * BL])
    y_p = param("y", [YD, T_total * BL])
    h0_p = param("h0f", [HD, 2 * BL])
    w_shapes = dict(
        wih0T=(HD, 3 * HD), whh0T=(HD, 3 * HD),
        wih1T=(HD, 3 * HD), whh1T=(HD, 3 * HD),
        pw0T=(UD, HD), pw1T=(HD, HD),
        dw0aT=(HD, HD), dw0bT=(HD, HD), dw1T=(HD, HD),
        xmvT=(HD, 2 * ZD),
        px0T=(2 * ZD, HD), px1T=(HD, HD),
        mw0T=(HD, HD), mw1T=(HD, YD), CT=(ZD, YD),
    )
    b_shapes = dict(
        pb0=(HD, 1), pb1=(HD, 1), db0=(HD, 1), db1=(HD, 1),
        xmvb=(2 * ZD, 1), pxb0=(HD, 1), pxb1=(HD, 1),
        mb0=(HD, 1), mb1=(YD, 1),
    )
    w_params = {n: param(n, s) for n, s in w_shapes.items()}
    b_params = {n: param(n, s, F32) for n, s in b_shapes.items()}
    out_p = nc.declare_dram_parameter("out", [YD, 2 * NGT], F32, isOutput=True)

    with tile.TileContext(nc) as tc:
        with (
            tc.tile_pool(name="const", bufs=1) as const,
            tc.tile_pool(name="big", bufs=1) as big,
            tc.tile_pool(name="io", bufs=2) as io,
            tc.tile_pool(name="scan", bufs=3) as scan,
            tc.tile_pool(name="meas", bufs=2) as meas,
            tc.tile_pool(name="psg_p", bufs=3, space="PSUM") as psg_p,
            tc.tile_pool(name="psm_p", bufs=3, space="PSUM") as psm_p,
            tc.tile_pool(name="psb_p", bufs=2, space="PSUM") as psb_p,
        ):
            mm = nc.tensor.matmul
            act = nc.scalar.activation
            dma = nc.sync.dma_start

            # ---- constants into SBUF ----
            W = {}
            for n, s in w_shapes.items():
                W[n] = const.tile(list(s), BF, tag=n, name=n)
                dma(W[n][:], w_params[n][:])
            BI = {}
            for n, s in b_shapes.items():
                BI[n] = const.tile(list(s), F32, tag=n, name="b_" + n)
                dma(BI[n][:], b_params[n][:])

            # ---- persistent state ----
            h0ring = big.tile([HD, 5 * BL], BF, tag="h0ring", name="h0ring")
            h_hist = big.tile([HD, (Tc + 1) * BL], BF, tag="h_hist", name="h_hist")
            phis = [big.tile([HD, Tc * BL], BF, tag=f"phi{i}", name=f"phi{i}") for i in range(2)]
            sq_acc = big.tile([YD, NGT], F32, tag="sq_acc", name="sq_acc")
            xy_acc = big.tile([YD, NGT], F32, tag="xy_acc", name="xy_acc")

            dma(h0ring[:, 0:BL], h0_p[:, 0:BL])
            dma(h_hist[:, 0:BL], h0_p[:, BL:2 * BL])

            def chunk_cols(ci):
                return slice(ci * Tc * BL, (ci + 1) * Tc * BL)

            def load_uy(ci):
                uc = io.tile([UD, Tc * BL], BF, tag="u", name="u")
                dma(uc[:], u_p[:, chunk_cols(ci)])
                yc = io.tile([YD, Tc * BL], BF, tag="y", name="y")
                dma(yc[:], y_p[:, chunk_cols(ci)])
                return uc, yc

            def emit_bulk_group(uc, phid, g):
                """phi_u for one group of 4 timesteps (512 cols)."""
                cs = slice(g * GW, (g + 1) * GW)
                ps1 = psb_p.tile([HD, GW], F32, tag="pb", name="pb")
                mm(ps1[:], W["pw0T"][:], uc[:, cs], start=True, stop=True)
                hid = scan.tile([HD, GW], BF, tag="phid", name="phid")
                nc.vector.tensor_scalar(hid[:], ps1[:], BI["pb0"][:], 0.0,
                                        op0=OP.add, op1=OP.max)
                ps2 = psb_p.tile([HD, GW], F32, tag="pb", name="pb")
                mm(ps2[:], W["pw1T"][:], hid[:], start=True, stop=True)
                nc.vector.tensor_scalar(phid[:, cs], ps2[:], BI["pb1"][:], None,
                                        op0=OP.add)

            def emit_gru_layer(wi, wh, x_ap, h_ap, n_dst, hn_pair, h_dst):
                """One GRU layer step.  x_ap: input [HD,BL]; h_ap: prev h.
                n_dst: where tanh n goes (must be the col right after h in
                hn_pair).  hn_pair: [HD, 2*BL] = [h_prev | n].  h_dst: where
                the new h goes (same col range as n_dst)."""
                psg = psg_p.tile([HD, 4 * BL], F32, tag="psg", name="psg")
                mm(psg[:, 0:BL], W[wi][:, 0:HD], x_ap, start=True, stop=False)
                mm(psg[:, BL:2 * BL], W[wi][:, HD:2 * HD], x_ap, start=True, stop=False)
                mm(psg[:, 2 * BL:3 * BL], W[wi][:, 2 * HD:3 * HD], x_ap,
                   start=True, stop=True)
                mm(psg[:, 0:BL], W[wh][:, 0:HD], h_ap, start=False, stop=True)
                mm(psg[:, BL:2 * BL], W[wh][:, HD:2 * HD], h_ap, start=False, stop=True)
                mm(psg[:, 3 * BL:4 * BL], W[wh][:, 2 * HD:3 * HD], h_ap,
                   start=True, stop=True)
                rA = scan.tile([HD, 3 * BL], BF, tag="rA", name="rA")
                act(rA[:, 0:2 * BL], psg[:, 0:2 * BL], AF.Sigmoid)
                act(rA[:, 2 * BL:3 * BL], psg[:, BL:2 * BL], AF.Sigmoid, scale=-1.0)
                t0 = scan.tile([HD, BL], BF, tag="t0", name="t0")
                nc.vector.tensor_mul(t0[:], rA[:, 0:BL], psg[:, 3 * BL:4 * BL])
                m0 = scan.tile([HD, BL], BF, tag="m0", name="m0")
                nc.vector.tensor_add(m0[:], t0[:], psg[:, 2 * BL:3 * BL])
                act(n_dst, m0[:], AF.Tanh)
                ab = scan.tile([HD, 2 * BL], BF, tag="ab", name="ab")
                nc.vector.tensor_mul(ab[:], rA[:, BL:3 * BL], hn_pair)
                nc.vector.tensor_add(h_dst, ab[:, 0:BL], ab[:, BL:2 * BL])

            def emit_step(ci, tl, phic):
                t = ci * Tc + tl
                k = t % 4
                x0 = phic[:, tl * BL:(tl + 1) * BL]
                emit_gru_layer(
                    "wih0T", "whh0T", x0,
                    h0ring[:, k * BL:(k + 1) * BL],
                    h0ring[:, (k + 1) * BL:(k + 2) * BL],
                    h0ring[:, k * BL:(k + 2) * BL],
                    h0ring[:, (k + 1) * BL:(k + 2) * BL],
                )
                if k == 3:
                    nc.vector.tensor_copy(h0ring[:, 0:BL], h0ring[:, 4 * BL:5 * BL])
                x1 = h0ring[:, (k + 1) * BL:(k + 2) * BL]
                emit_gru_layer(
                    "wih1T", "whh1T", x1,
                    h_hist[:, tl * BL:(tl + 1) * BL],
                    h_hist[:, (tl + 1) * BL:(tl + 2) * BL],
                    h_hist[:, tl * BL:(tl + 2) * BL],
                    h_hist[:, (tl + 1) * BL:(tl + 2) * BL],
                )

            def emit_meas(ci, g, phic, yc):
                cs = slice(g * GW, (g + 1) * GW)
                psA = psm_p.tile([HD, GW], F32, tag="pm", name="pm")
                mm(psA[:], W["dw0aT"][:], phic[:, cs], start=True, stop=False)
                mm(psA[:], W["dw0bT"][:], h_hist[:, cs], start=False, stop=True)
                dh = meas.tile([HD, GW], BF, tag="dh", name="dh")
                nc.vector.tensor_scalar(dh[:], psA[:], BI["db0"][:], 0.0,
                                        op0=OP.add, op1=OP.max)
                psB = psm_p.tile([HD, GW], F32, tag="pm", name="pm")
                mm(psB[:], W["dw1T"][:], dh[:], start=True, stop=True)
                dphi = meas.tile([HD, GW], BF, tag="dphi", name="dphi")
                act(dphi[:], psB[:], AF.Identity, bias=BI["db1"][:])
                psC = psm_p.tile([2 * ZD, GW], F32, tag="pm", name="pm")
                mm(psC[:], W["xmvT"][:], dphi[:], start=True, stop=True)
                xmv = meas.tile([2 * ZD, GW], BF, tag="xmv", name="xmv")
                nc.vector.tensor_scalar(xmv[:], psC[:], BI["xmvb"][:], None,
                                        op0=OP.add)
                psD = psm_p.tile([HD, GW], F32, tag="pm", name="pm")
                mm(psD[:], W["px0T"][:], xmv[:], start=True, stop=True)
                ph = meas.tile([HD, GW], BF, tag="ph", name="ph")
                act(ph[:], psD[:], AF.Relu, bias=BI["pxb0"][:])
                psE = psm_p.tile([HD, GW], F32, tag="pm", name="pm")
                mm(psE[:], W["px1T"][:], ph[:], start=True, stop=True)
                px = meas.tile([HD, GW], BF, tag="px", name="px")
                nc.vector.tensor_scalar(px[:], psE[:], BI["pxb1"][:], None,
                                        op0=OP.add)
                psF = psm_p.tile([HD, GW], F32, tag="pm", name="pm")
                mm(psF[:], W["mw0T"][:], px[:], start=True, stop=True)
                mh = meas.tile([HD, GW], BF, tag="mh", name="mh")
                act(mh[:], psF[:], AF.Relu, bias=BI["mb0"][:])
                psY = psm_p.tile([YD, GW], F32, tag="pm", name="pm")
                mm(psY[:], W["mw1T"][:], mh[:], start=True, stop=False)
                mm(psY[:], W["CT"][:], xmv[0:ZD, :], start=False, stop=True)
                gg = ci * NG + g
                jA = meas.tile([YD, GW], F32, tag="jA", name="jA")
                act(jA[:], psY[:], AF.Square, bias=BI["mb1"][:],
                    accum_out=sq_acc[:, gg:gg + 1])
                jB = meas.tile([YD, GW], F32, tag="jB", name="jB")
                nc.vector.affine_mul_reduce(jB[:], xy_acc[:, gg:gg + 1],
                                            psY[:], yc[:, cs], 1.0, BI["mb1"][:])

            # ---- main schedule ----
            uc, yc = load_uy(0)
            for g in range(NG):
                emit_bulk_group(uc, phis[0], g)
            uy_next = load_uy(1) if NCH > 1 else None
            for ci in range(NCH):
                phic = phis[ci % 2]
                uc_n = uy_next[0] if uy_next is not None else None
                yc_cur = yc
                for tl in range(Tc):
                    emit_step(ci, tl, phic)
                    if tl % 4 == 3:
                        g = tl // 4
                        emit_meas(ci, g, phic, yc_cur)
                        if uc_n is not None:
                            emit_bulk_group(uc_n, phis[(ci + 1) % 2], g)
                # h1 carry: slot Tc -> slot 0
                nc.vector.tensor_copy(h_hist[:, 0:BL],
                                      h_hist[:, Tc * BL:(Tc + 1) * BL])
                if uy_next is not None:
                    yc = uy_next[1]
                    uy_next = load_uy(ci + 2) if ci + 2 < NCH else None

            dma(out_p[:, 0:NGT], sq_acc[:])
            dma(out_p[:, NGT:2 * NGT], xy_acc[:])

    nc.compile()
    meta = dict(T_total=T_total, Tc=Tc, NGT=NGT)
    return nc, meta


def prep_inputs(inputs, T_total=T):
    """Host-side shard + relayout.  Returns (in_maps, y_sq_sum)."""
    u = np.asarray(inputs["u"], np.float32)
    y = np.asarray(inputs["y"], np.float32)
    h0 = np.asarray(inputs["h0"], np.float32)
    if T_total != T:
        u = u[:, :, :T_total]
        y = y[:, :, :T_total]

    g = lambda n: np.asarray(inputs[n], np.float32)
    wih, whh = g("gru_wih"), g("gru_whh")
    shared = {
        "wih0T": wih[0].T, "whh0T": whh[0].T,
        "wih1T": wih[1].T, "whh1T": whh[1].T,
        "pw0T": g("phi_u_w0").T, "pw1T": g("phi_u_w1").T,
        "dw0aT": g("dynn_w0")[:, :HD].T, "dw0bT": g("dynn_w0")[:, HD:].T,
        "dw1T": g("dynn_w1").T,
        "xmvT": np.concatenate([g("x_mean_w"), g("x_logvar_w")], 0).T,
        "px0T": g("phi_x_w0").T, "px1T": g("phi_x_w1").T,
        "mw0T": g("menn_w0").T, "mw1T": g("menn_w1").T, "CT": g("C").T,
    }
    shared = {k: np.ascontiguousarray(v, dtype=BF_NP) for k, v in shared.items()}
    biases = {
        "pb0": g("phi_u_b0"), "pb1": g("phi_u_b1"),
        "db0": g("dynn_b0"), "db1": g("dynn_b1"),
        "xmvb": np.concatenate([g("x_mean_b"), g("x_logvar_b")], 0),
        "pxb0": g("phi_x_b0"), "pxb1": g("phi_x_b1"),
        "mb0": g("menn_b0"), "mb1": g("menn_b1"),
    }
    for k, v in biases.items():
        shared[k] = np.ascontiguousarray(v[:, None], dtype=np.float32)

    in_maps = []
    for c in range(NCORES):
        bs = slice(c * BL, (c + 1) * BL)
        ub = u[bs]                      # (BL, UD, Tt)
        yb = y[bs]
        m = dict(shared)
        m["u"] = np.ascontiguousarray(
            ub.transpose(1, 2, 0).reshape(UD, -1), dtype=BF_NP)
        m["y"] = np.ascontiguousarray(
            yb.transpose(1, 2, 0).reshape(YD, -1), dtype=BF_NP)
        h0b = h0[:, bs, :]              # (L, BL, HD)
        m["h0f"] = np.ascontiguousarray(
            np.concatenate([h0b[0].T, h0b[1].T], 1), dtype=BF_NP)
        in_maps.append(m)

    y_sq = float(np.dot(y.reshape(-1).astype(np.float64),
                        y.reshape(-1).astype(np.float64)))
    return in_maps, y_sq


def reduce_outputs(results, meta, y_sq):
    NGT = meta["NGT"]
    total = 0.0
    for r in results:
        o = np.asarray(r["out"], np.float64)
        total += o[:, :NGT].sum() - 2.0 * o[:, NGT:].sum()
    return np.float32(total + y_sq)


_CACHE = {}


def kernel(**inputs):
    key = ("full", T, 64)
    if key not in _CACHE:
        _CACHE[key] = build(T, 64)
    nc, meta = _CACHE[key]
    in_maps, y_sq = prep_inputs(inputs, T)
    res = run_bass_kernel_spmd(nc, in_maps, core_ids=list(range(NCORES)))
    return reduce_outputs(res.results, meta, y_sq)
